# revision 20
# baseline (speedup 1.0000x reference)
# Multi-head attention (B=4, N=2048, D=1024, H=16, DH=64, OUT=1024) on 8 TRN2 NeuronCores.
#
# Sharding: 4 groups x 2 cores. Core c -> batch c//2, head-half c%2 (8 heads).
# Wq/Wk/Wv column-split per head group, Wo row-split; host sums the two
# partial outputs per batch and adds the combined bias (bo + bv@Wo --
# exact because softmax rows sum to 1, so the V bias commutes through
# attention).
#
# Per-core kernel (bf16 compute, fp32 PSUM):
#   qT/kT projections in transposed layout [head_cols(128=2 heads), rows].
#   v projection in natural layout (no bias, no ones column).
#   Attention processes kcc PAIRS: one 4-bank PSUM tile [128, 2048] holds
#   scoresT for 2 k-chunks x 2 heads; ONE exp activation per pair (halves
#   ScalarE per-call overhead). AV uses COLUMN-TILED matmul pairs
#   (tile_position (0,0)/(0,64)): both heads' M=64 AV run concurrently in
#   disjoint PE column groups with independent streams -> 2x AV throughput
#   vs the M=65 ones-column scheme.
#   Softmax denominators (lost with the ones column) come from a pairwise
#   bf16 add tree over the exp tiles (DVE 2x mode, a few adds on GPSIMD)
#   + one tiny ones-matmul partition reduce per (g, qc) into PSUM.
#   Normalization: reciprocal + gpsimd partition_broadcast of both head
#   denominator rows + DVE 2x multiplies straight into ctxT partitions
#   (hi head lands on partitions 64:128 -- no bounce DMA).
#   Output projection contracts ctxT over head dims; raw psum is copied by
#   ScalarE to bf16 and DMA'd out; host adds biases and sums partials.
#
# kT/qT projections for groups 1-3 are emitted inside the attention loop so
# their PE work fills ScalarE-paced attention windows.

import contextlib

import numpy as np
import ml_dtypes

B, N, D, H = 4, 2048, 1024, 16
DH = D // H
OUT = 1024
NCORES = 8
KC = D // 128     # 8 contraction chunks for projections
RC = N // 128     # 16 row chunks
KCC2 = RC // 2    # 8 k-chunk pairs in attention
QC = N // 512     # 4 qrow chunks of 512
G = 4             # head-pair groups per core (8 heads / 2)
HPC = H // 2      # heads per core

GPSIMD_TREE = ()   # which of the 8 pair-sums run on gpsimd

_cache = {}


def _build_module(reps=1):
    import concourse.mybir as mybir
    import concourse.tile as tile
    from concourse import bacc

    bf16 = mybir.dt.bfloat16
    f32 = mybir.dt.float32
    Exp = mybir.ActivationFunctionType.Exp
    MULT = mybir.AluOpType.mult
    ADD = mybir.AluOpType.add

    nc = bacc.Bacc(trn_type="TRN2", target_bir_lowering=False)

    xt_q = nc.declare_dram_parameter("xt_q", [KC, 128, N], bf16, isOutput=False)
    xt_k = nc.declare_dram_parameter("xt_k", [KC, 128, N], bf16, isOutput=False)
    xt_v = nc.declare_dram_parameter("xt_v", [KC, 128, N], bf16, isOutput=False)
    wq_d = nc.declare_dram_parameter("wq", [KC, 128, 512], bf16, isOutput=False)
    wk_d = nc.declare_dram_parameter("wk", [KC, 128, 512], bf16, isOutput=False)
    wv_d = nc.declare_dram_parameter("wv", [KC, 128, 512], bf16, isOutput=False)
    wo_d = nc.declare_dram_parameter("wo", [G, 128, OUT], bf16, isOutput=False)
    bq_d = nc.declare_dram_parameter("bq2", [G, 128, 1], f32, isOutput=False)
    bk_d = nc.declare_dram_parameter("bk2", [G, 128, 1], f32, isOutput=False)
    out_d = nc.declare_dram_parameter("out", [N, OUT], bf16, isOutput=True)

    with tile.TileContext(nc) as tc, contextlib.ExitStack() as ctx:
        weights = ctx.enter_context(tc.tile_pool(name="weights", bufs=1))
        qkv = ctx.enter_context(tc.tile_pool(name="qkv", bufs=1))
        xt_pool = ctx.enter_context(tc.tile_pool(name="xt", bufs=16))
        expp = ctx.enter_context(tc.tile_pool(name="expp", bufs=5))
        treep = ctx.enter_context(tc.tile_pool(name="treep", bufs=2))
        ctxp = ctx.enter_context(tc.tile_pool(name="ctxp", bufs=1))
        small = ctx.enter_context(tc.tile_pool(name="small", bufs=4))
        outp = ctx.enter_context(tc.tile_pool(name="outp", bufs=3))
        ps_proj = ctx.enter_context(tc.tile_pool(name="ps_proj", bufs=2, space="PSUM"))
        ps_qk = ctx.enter_context(tc.tile_pool(name="ps_qk", bufs=2, space="PSUM"))
        ps_av = ctx.enter_context(tc.tile_pool(name="ps_av", bufs=1, space="PSUM"))

        for rep in range(reps):
            # ---- compute-critical DMAs first: v weights + xt_v, then xt_k
            # streams in during the v projection, then the remaining weights.
            wv_sb = weights.tile([128, KC, 512], bf16, tag="wvo")
            for kc in range(KC):
                nc.sync.dma_start(wv_sb[:, kc, :], wv_d[kc])
            xtv = []
            for kc in range(KC):
                t = xt_pool.tile([128, N], bf16, tag="xt")
                nc.sync.dma_start(t[:], xt_v[kc])
                xtv.append(t)
            wk_sb = weights.tile([128, KC, 512], bf16, tag="wk")
            for kc in range(KC):
                nc.sync.dma_start(wk_sb[:, kc, :], wk_d[kc])
            bk_sb = weights.tile([128, G, 1], f32, tag="bk")
            nc.sync.dma_start(bk_sb[:], bk_d.rearrange("g p o -> p g o"))
            ones_sb = weights.tile([128, 1], bf16, tag="ones")
            nc.vector.memset(ones_sb[:], 1.0)

            # ---- V projection: v1[rc] = x@Wv slice, bf16 [128, 8 heads * 64]
            # (no bias: bv commutes through softmax into the host-side bo)
            v1 = []
            for rc in range(RC):
                ps = ps_proj.tile([128, 512], f32, tag="pp")
                for kc in range(KC):
                    nc.tensor.matmul(
                        ps[:],
                        xtv[kc][:, rc * 128:(rc + 1) * 128],
                        wv_sb[:, kc, :],
                        start=(kc == 0), stop=(kc == KC - 1),
                    )
                t = qkv.tile([128, 512], bf16, tag=f"v1_{rc}")
                nc.scalar.copy(t[:], ps[:])
                v1.append(t)

            # ---- K^T projection for all 4 pair-groups: kT[g] [128(2 heads x 64), N]
            xtk = []
            for kc in range(KC):
                t = xt_pool.tile([128, N], bf16, tag="xt")
                nc.sync.dma_start(t[:], xt_k[kc])
                xtk.append(t)
            wq_sb = weights.tile([128, KC, 512], bf16, tag="wq")
            for kc in range(KC):
                nc.sync.dma_start(wq_sb[:, kc, :], wq_d[kc])
            bq_sb = weights.tile([128, G, 1], f32, tag="bq")
            nc.sync.dma_start(bq_sb[:], bq_d.rearrange("g p o -> p g o"))

            def emit_kT(g):
                t = qkv.tile([128, N], bf16, tag="kT", bufs=2, name=f"kT_{g}")
                for qn2 in range(0, QC, 2):
                    pss = [ps_proj.tile([128, 512], f32, tag="pp", name=f"pk{g}{qn2}{j}")
                           for j in range(2)]
                    for kc in range(KC):
                        for j in range(2):
                            nc.tensor.matmul(
                                pss[j][:],
                                wk_sb[:, kc, g * 128:(g + 1) * 128],
                                xtk[kc][:, (qn2 + j) * 512:(qn2 + j + 1) * 512],
                                start=(kc == 0), stop=(kc == KC - 1),
                            )
                    for j in range(2):
                        nc.scalar.add(
                            t[:, (qn2 + j) * 512:(qn2 + j + 1) * 512],
                            pss[j][:], bk_sb[:, g, :]
                        )
                return t

            kT = [None] * G
            kT[0] = emit_kT(0)

            # ---- per pair-group: Q^T projection then attention
            xtq = []
            for kc in range(KC):
                t = xt_pool.tile([128, N], bf16, tag="xt")
                nc.sync.dma_start(t[:], xt_q[kc])
                xtq.append(t)

            # wo reuses wv's buffer (wv is dead after the V projection)
            wo_sb = weights.tile([128, G, OUT], bf16, tag="wvo")
            for g in range(G):
                nc.sync.dma_start(wo_sb[:, g, :], wo_d[g])
            ctxT = [
                ctxp.tile([128, N], bf16, tag=f"ctxT_{g}", name=f"ctxT_{g}")
                for g in range(G)
            ]
            for g in range(G):
                if g > 0:
                    kT[g] = emit_kT(g)
                qT = qkv.tile([128, N], bf16, tag="qT", bufs=2, name=f"qT_{g}")
                for qn2 in range(0, QC, 2):
                    pss = [ps_proj.tile([128, 512], f32, tag="pp", name=f"pq{g}{qn2}{j}")
                           for j in range(2)]
                    for kc in range(KC):
                        for j in range(2):
                            nc.tensor.matmul(
                                pss[j][:],
                                wq_sb[:, kc, g * 128:(g + 1) * 128],
                                xtq[kc][:, (qn2 + j) * 512:(qn2 + j + 1) * 512],
                                start=(kc == 0), stop=(kc == KC - 1),
                            )
                    for j in range(2):
                        nc.scalar.add(
                            qT[:, (qn2 + j) * 512:(qn2 + j + 1) * 512],
                            pss[j][:], bq_sb[:, g, :]
                        )

                lo = slice(2 * g * DH, (2 * g + 1) * DH)
                hi = slice((2 * g + 1) * DH, (2 * g + 2) * DH)
                for qc in range(QC):
                    av = ps_av.tile([128, 512], f32, tag="av")
                    eTs = [None] * RC
                    s1 = [None] * KCC2
                    s2 = [None] * 4
                    s3 = [None] * 2
                    for kcc in range(RC):
                        pp = ps_qk.tile([128, 1024], f32, tag="qk")
                        nc.tensor.matmul(
                            pp[:, 0:512],
                            kT[g][0:64, kcc * 128:(kcc + 1) * 128],
                            qT[0:64, qc * 512:(qc + 1) * 512],
                            start=True, stop=True,
                        )
                        nc.tensor.matmul(
                            pp[:, 512:1024],
                            kT[g][64:128, kcc * 128:(kcc + 1) * 128],
                            qT[64:128, qc * 512:(qc + 1) * 512],
                            start=True, stop=True,
                        )
                        eT = expp.tile([128, 1024], bf16, tag="exp")
                        eTs[kcc] = eT
                        nc.scalar.activation(eT[:], pp[:], Exp)
                        nc.tensor.matmul(
                            av[0:64, :],
                            v1[kcc][:, lo],
                            eT[:, 0:512],
                            start=(kcc == 0), stop=(kcc == RC - 1),
                            tile_position=(0, 0),
                        )
                        nc.tensor.matmul(
                            av[64:128, :],
                            v1[kcc][:, hi],
                            eT[:, 512:1024],
                            start=(kcc == 0), stop=(kcc == RC - 1),
                            tile_position=(0, 64),
                        )
                        # denominator tree (bf16 adds, DVE 2x mode)
                        if kcc % 2 == 1:
                            k2 = kcc // 2
                            s1[k2] = treep.tile([128, 1024], bf16, tag="s1",
                                                name=f"s1_{g}{qc}{k2}")
                            nc.vector.tensor_tensor(
                                s1[k2][:], eTs[kcc - 1][:], eT[:], ADD)
                            if k2 % 2 == 1:
                                i = k2 // 2
                                s2[i] = treep.tile([128, 1024], bf16, tag="s2",
                                                   name=f"s2_{g}{qc}{i}")
                                nc.vector.tensor_tensor(
                                    s2[i][:], s1[k2 - 1][:], s1[k2][:], ADD)
                                if i % 2 == 1:
                                    s3[i // 2] = treep.tile(
                                        [128, 1024], bf16, tag="s3",
                                        name=f"s3_{g}{qc}{i // 2}")
                                    nc.vector.tensor_tensor(
                                        s3[i // 2][:], s2[i - 1][:], s2[i][:],
                                        ADD)
                    root = treep.tile([128, 1024], bf16, tag="root",
                                      name=f"root_{g}{qc}")
                    nc.vector.tensor_tensor(root[:], s3[0][:], s3[1][:], ADD)

                    # denominators: ones-matmul partition reduce (one bank)
                    d2 = ps_av.tile([33, 512], f32, tag="d2")
                    nc.tensor.matmul(d2[0:1, :], ones_sb[:], root[:, 0:512],
                                     start=True, stop=True,
                                     tile_position=(0, 0))
                    nc.tensor.matmul(d2[32:33, :], ones_sb[:], root[:, 512:1024],
                                     start=True, stop=True,
                                     tile_position=(0, 32))
                    # raw ctx out of PSUM (ScalarE; frees the AV bank)
                    raw = small.tile([128, 512], bf16, tag="raw", bufs=2)
                    nc.scalar.copy(raw[:], av[:])
                    r = small.tile([1, 1024], bf16, tag="r", bufs=2)
                    with nc.allow_low_precision(
                            reason="softmax denom: positive sums, bf16 ok"):
                        nc.vector.reciprocal(r[:, 0:512], d2[0:1, :])
                        nc.vector.reciprocal(r[:, 512:1024], d2[32:33, :])
                    rb = small.tile([128, 1024], bf16, tag="rb", bufs=2)
                    nc.gpsimd.partition_broadcast(rb[:], r[:])
                    nc.vector.tensor_tensor(
                        ctxT[g][0:64, qc * 512:(qc + 1) * 512],
                        raw[0:64, :], rb[0:64, 0:512], MULT,
                    )
                    nc.vector.tensor_tensor(
                        ctxT[g][64:128, qc * 512:(qc + 1) * 512],
                        raw[64:128, :], rb[64:128, 512:1024], MULT,
                    )

            # ---- output projection: out = ctx @ Wo_slice (biases on host)
            for rc in range(RC):
                pss = [ps_proj.tile([128, 512], f32, tag="pp", name=f"po{rc}{n}")
                       for n in range(2)]
                for g in range(G):
                    for ncol in range(2):
                        nc.tensor.matmul(
                            pss[ncol][:],
                            ctxT[g][:, rc * 128:(rc + 1) * 128],
                            wo_sb[:, g, ncol * 512:(ncol + 1) * 512],
                            start=(g == 0), stop=(g == G - 1),
                        )
                for ncol in range(2):
                    ob = outp.tile([128, 512], bf16, tag="ob")
                    nc.scalar.copy(ob[:], pss[ncol][:])
                    nc.sync.dma_start(
                        out_d[rc * 128:(rc + 1) * 128,
                              ncol * 512:(ncol + 1) * 512], ob[:]
                    )

    nc.compile()
    return nc


def _get_module(reps=1):
    key = ("nc", reps)
    if key not in _cache:
        _cache[key] = _build_module(reps)
    return _cache[key]


def _get_runner(reps=1, donate=True):
    """Build the PJRT executable once (mirrors bass2jax.run_bass_via_pjrt) and
    return a callable in_maps -> list of per-core output dicts."""
    rkey = ("runner", reps, donate)
    if rkey in _cache:
        return _cache[rkey]

    import jax
    import numpy as np
    import concourse.mybir as mybir
    from concourse import bass2jax
    from jax.sharding import Mesh, PartitionSpec
    from jax.experimental.shard_map import shard_map

    nc = _get_module(reps)
    bass2jax.install_neuronx_cc_hook()

    partition_name = nc.partition_id_tensor.name if nc.partition_id_tensor else None
    in_names, out_names, out_avals, zero_outs = [], [], [], []
    for alloc in nc.m.functions[0].allocations:
        if not isinstance(alloc, mybir.MemoryLocationSet):
            continue
        name = alloc.memorylocations[0].name
        if alloc.kind == "ExternalInput":
            if name != partition_name:
                in_names.append(name)
        elif alloc.kind == "ExternalOutput":
            shape = tuple(alloc.tensor_shape)
            dtype = mybir.dt.np(alloc.dtype)
            out_names.append(name)
            out_avals.append(jax.core.ShapedArray(shape, dtype))
            zero_outs.append(np.zeros(shape, dtype))
    n_params = len(in_names)
    n_outs = len(out_avals)
    all_in_names = list(in_names) + list(out_names)
    if partition_name is not None:
        all_in_names.append(partition_name)
    donate_idx = tuple(range(n_params, n_params + n_outs))

    def _body(*args):
        operands = list(args)
        if partition_name is not None:
            operands.append(bass2jax.partition_id_tensor())
        outs = bass2jax._bass_exec_p.bind(
            *operands,
            out_avals=tuple(out_avals),
            in_names=tuple(all_in_names),
            out_names=tuple(out_names),
            lowering_input_output_aliases=(),
            sim_require_finite=True,
            sim_require_nnan=True,
            nc=nc,
        )
        return tuple(outs)

    devices = jax.devices()[:NCORES]
    mesh = Mesh(np.asarray(devices), ("core",))
    in_specs = (PartitionSpec("core"),) * (n_params + n_outs)
    out_specs = (PartitionSpec("core"),) * n_outs
    sharded = jax.jit(
        shard_map(_body, mesh=mesh, in_specs=in_specs, out_specs=out_specs,
                  check_rep=False),
        donate_argnums=(donate_idx if donate else ()), keep_unused=True,
    )

    def run(in_maps):
        concat_in = [
            np.concatenate([np.asarray(in_maps[c][name]) for c in range(NCORES)], axis=0)
            for name in in_names
        ]
        concat_zeros = [
            np.zeros((NCORES * z.shape[0], *z.shape[1:]), z.dtype) for z in zero_outs
        ]
        out_arrs = sharded(*concat_in, *concat_zeros)
        return [
            {
                name: np.asarray(out_arrs[i]).reshape(NCORES, *out_avals[i].shape)[c]
                for i, name in enumerate(out_names)
            }
            for c in range(NCORES)
        ]

    run.in_names = in_names
    run.out_names = out_names
    run.out_avals = out_avals
    run.zero_outs = zero_outs
    run.sharded = sharded
    _cache[rkey] = run
    return run


def _shard_inputs(key, value, query, Wk, bk, Wv, bv, Wq, bq, Wo, bo):
    bf = ml_dtypes.bfloat16
    f32 = np.float32
    scale = 1.0 / np.sqrt(np.float32(DH))

    xt = {}  # per batch transposed inputs
    for b in range(B):
        xt[b] = {
            "q": np.ascontiguousarray(query[b].T).reshape(KC, 128, N).astype(bf),
            "k": np.ascontiguousarray(key[b].T).reshape(KC, 128, N).astype(bf),
            "v": np.ascontiguousarray(value[b].T).reshape(KC, 128, N).astype(bf),
        }

    in_maps = []
    for c in range(NCORES):
        b, half = divmod(c, 2)
        cols = slice(half * 512, (half + 1) * 512)
        in_maps.append({
            "xt_q": xt[b]["q"],
            "xt_k": xt[b]["k"],
            "xt_v": xt[b]["v"],
            "wq": np.ascontiguousarray(Wq[:, cols] * scale).reshape(KC, 128, 512).astype(bf),
            "wk": np.ascontiguousarray(Wk[:, cols]).reshape(KC, 128, 512).astype(bf),
            "wv": np.ascontiguousarray(Wv[:, cols]).reshape(KC, 128, 512).astype(bf),
            "wo": np.ascontiguousarray(Wo[cols, :]).reshape(G, 128, OUT).astype(bf),
            "bq2": (bq[cols] * scale).reshape(G, 128, 1).astype(f32),
            "bk2": bk[cols].reshape(G, 128, 1).astype(f32),
        })
    return in_maps


def kernel(key, value, query, Wk, bk, Wv, bv, Wq, bq, Wo, bo):
    key, value, query = np.asarray(key), np.asarray(value), np.asarray(query)
    Wk, bk, Wv, bv = np.asarray(Wk), np.asarray(bk), np.asarray(Wv), np.asarray(bv)
    Wq, bq, Wo, bo = np.asarray(Wq), np.asarray(bq), np.asarray(Wo), np.asarray(bo)

    run = _get_runner()
    in_maps = _shard_inputs(key, value, query, Wk, bk, Wv, bv, Wq, bq, Wo, bo)
    results = run(in_maps)
    # bv commutes through softmax (attention rows sum to 1): fold into bias.
    bo_total = (bo.astype(np.float64) + bv.astype(np.float64) @ Wo.astype(np.float64)
                ).astype(np.float32)
    out = np.empty((B, N, OUT), np.float32)
    for b in range(B):
        out[b] = (results[2 * b]["out"].astype(np.float32)
                  + results[2 * b + 1]["out"].astype(np.float32)
                  + bo_total)
    return out


# revision 22
# speedup vs baseline: 2.1938x; 2.1938x over previous
# Multi-head attention (B=4, N=2048, D=1024, H=16, DH=64, OUT=1024) on 8 TRN2 NeuronCores.
#
# Sharding: 4 groups x 2 cores. Core c -> batch c//2, head-half c%2 (8 heads).
# Wq/Wk/Wv column-split per head group, Wo row-split; host sums the two
# partial outputs per batch and adds the combined bias (bo + bv@Wo --
# exact because softmax rows sum to 1, so the V bias commutes through
# attention).
#
# Per-core kernel (bf16 compute, fp32 PSUM):
#   qT/kT projections in transposed layout [head_cols(128=2 heads), rows].
#   v projection in natural layout (no bias, no ones column).
#   Attention processes kcc PAIRS: one 4-bank PSUM tile [128, 2048] holds
#   scoresT for 2 k-chunks x 2 heads; ONE exp activation per pair (halves
#   ScalarE per-call overhead). AV uses COLUMN-TILED matmul pairs
#   (tile_position (0,0)/(0,64)): both heads' M=64 AV run concurrently in
#   disjoint PE column groups with independent streams -> 2x AV throughput
#   vs the M=65 ones-column scheme.
#   Softmax denominators (lost with the ones column) come from a pairwise
#   bf16 add tree over the exp tiles (DVE 2x mode, a few adds on GPSIMD)
#   + one tiny ones-matmul partition reduce per (g, qc) into PSUM.
#   Normalization: reciprocal + gpsimd partition_broadcast of both head
#   denominator rows + DVE 2x multiplies straight into ctxT partitions
#   (hi head lands on partitions 64:128 -- no bounce DMA).
#   Output projection contracts ctxT over head dims; raw psum is copied by
#   ScalarE to bf16 and DMA'd out; host adds biases and sums partials.
#
# kT/qT projections for groups 1-3 are emitted inside the attention loop so
# their PE work fills ScalarE-paced attention windows.

import contextlib

import numpy as np
import ml_dtypes

B, N, D, H = 4, 2048, 1024, 16
DH = D // H
OUT = 1024
NCORES = 8
KC = D // 128     # 8 contraction chunks for projections
RC = N // 128     # 16 row chunks
KCC2 = RC // 2    # 8 k-chunk pairs in attention
QC = N // 512     # 4 qrow chunks of 512
G = 4             # head-pair groups per core (8 heads / 2)
HPC = H // 2      # heads per core

GPSIMD_TREE = ()   # which of the 8 pair-sums run on gpsimd

_cache = {}


def _build_module(reps=1):
    import concourse.mybir as mybir
    import concourse.tile as tile
    from concourse import bacc

    bf16 = mybir.dt.bfloat16
    f32 = mybir.dt.float32
    Exp = mybir.ActivationFunctionType.Exp
    MULT = mybir.AluOpType.mult
    ADD = mybir.AluOpType.add

    nc = bacc.Bacc(trn_type="TRN2", target_bir_lowering=False)

    xt_q = nc.declare_dram_parameter("xt_q", [KC, 128, N], bf16, isOutput=False)
    xt_k = nc.declare_dram_parameter("xt_k", [KC, 128, N], bf16, isOutput=False)
    xt_v = nc.declare_dram_parameter("xt_v", [KC, 128, N], bf16, isOutput=False)
    wq_d = nc.declare_dram_parameter("wq", [KC, 128, 512], bf16, isOutput=False)
    wk_d = nc.declare_dram_parameter("wk", [KC, 128, 512], bf16, isOutput=False)
    wv_d = nc.declare_dram_parameter("wv", [KC, 128, 512], bf16, isOutput=False)
    wo_d = nc.declare_dram_parameter("wo", [G, 128, OUT], bf16, isOutput=False)
    bq_d = nc.declare_dram_parameter("bq2", [G, 128, 1], f32, isOutput=False)
    bk_d = nc.declare_dram_parameter("bk2", [G, 128, 1], f32, isOutput=False)
    out_d = nc.declare_dram_parameter("out", [N, OUT], bf16, isOutput=True)

    with tile.TileContext(nc) as tc, contextlib.ExitStack() as ctx:
        weights = ctx.enter_context(tc.tile_pool(name="weights", bufs=1))
        qkv = ctx.enter_context(tc.tile_pool(name="qkv", bufs=1))
        xt_pool = ctx.enter_context(tc.tile_pool(name="xt", bufs=16))
        expp = ctx.enter_context(tc.tile_pool(name="expp", bufs=5))
        treep = ctx.enter_context(tc.tile_pool(name="treep", bufs=2))
        ctxp = ctx.enter_context(tc.tile_pool(name="ctxp", bufs=1))
        small = ctx.enter_context(tc.tile_pool(name="small", bufs=4))
        outp = ctx.enter_context(tc.tile_pool(name="outp", bufs=3))
        ps_proj = ctx.enter_context(tc.tile_pool(name="ps_proj", bufs=2, space="PSUM"))
        ps_qk = ctx.enter_context(tc.tile_pool(name="ps_qk", bufs=2, space="PSUM"))
        ps_av = ctx.enter_context(tc.tile_pool(name="ps_av", bufs=1, space="PSUM"))

        for rep in range(reps):
            # ---- compute-critical DMAs first: v weights + xt_v, then xt_k
            # streams in during the v projection, then the remaining weights.
            wv_sb = weights.tile([128, KC, 512], bf16, tag="wvo")
            for kc in range(KC):
                nc.sync.dma_start(wv_sb[:, kc, :], wv_d[kc])
            xtv = []
            for kc in range(KC):
                t = xt_pool.tile([128, N], bf16, tag="xt")
                nc.sync.dma_start(t[:], xt_v[kc])
                xtv.append(t)
            wk_sb = weights.tile([128, KC, 512], bf16, tag="wk")
            for kc in range(KC):
                nc.sync.dma_start(wk_sb[:, kc, :], wk_d[kc])
            bk_sb = weights.tile([128, G, 1], f32, tag="bk")
            nc.sync.dma_start(bk_sb[:], bk_d.rearrange("g p o -> p g o"))
            ones_sb = weights.tile([128, 64], bf16, tag="ones")
            nc.vector.memset(ones_sb[:], 1.0)

            # ---- V projection: v1[rc] = x@Wv slice, bf16 [128, 8 heads * 64]
            # (no bias: bv commutes through softmax into the host-side bo)
            v1 = []
            for rc in range(RC):
                ps = ps_proj.tile([128, 512], f32, tag="pp")
                for kc in range(KC):
                    nc.tensor.matmul(
                        ps[:],
                        xtv[kc][:, rc * 128:(rc + 1) * 128],
                        wv_sb[:, kc, :],
                        start=(kc == 0), stop=(kc == KC - 1),
                    )
                t = qkv.tile([128, 512], bf16, tag=f"v1_{rc}")
                nc.scalar.copy(t[:], ps[:])
                v1.append(t)

            # ---- K^T projection for all 4 pair-groups: kT[g] [128(2 heads x 64), N]
            xtk = []
            for kc in range(KC):
                t = xt_pool.tile([128, N], bf16, tag="xt")
                nc.sync.dma_start(t[:], xt_k[kc])
                xtk.append(t)
            wq_sb = weights.tile([128, KC, 512], bf16, tag="wq")
            for kc in range(KC):
                nc.sync.dma_start(wq_sb[:, kc, :], wq_d[kc])
            bq_sb = weights.tile([128, G, 1], f32, tag="bq")
            nc.sync.dma_start(bq_sb[:], bq_d.rearrange("g p o -> p g o"))

            def emit_kT(g):
                t = qkv.tile([128, N], bf16, tag="kT", bufs=2, name=f"kT_{g}")
                for qn2 in range(0, QC, 2):
                    pss = [ps_proj.tile([128, 512], f32, tag="pp", name=f"pk{g}{qn2}{j}")
                           for j in range(2)]
                    for kc in range(KC):
                        for j in range(2):
                            nc.tensor.matmul(
                                pss[j][:],
                                wk_sb[:, kc, g * 128:(g + 1) * 128],
                                xtk[kc][:, (qn2 + j) * 512:(qn2 + j + 1) * 512],
                                start=(kc == 0), stop=(kc == KC - 1),
                            )
                    for j in range(2):
                        nc.scalar.add(
                            t[:, (qn2 + j) * 512:(qn2 + j + 1) * 512],
                            pss[j][:], bk_sb[:, g, :]
                        )
                return t

            kT = [None] * G
            kT[0] = emit_kT(0)

            # ---- per pair-group: Q^T projection then attention
            xtq = []
            for kc in range(KC):
                t = xt_pool.tile([128, N], bf16, tag="xt")
                nc.sync.dma_start(t[:], xt_q[kc])
                xtq.append(t)

            # wo reuses wv's buffer (wv is dead after the V projection)
            wo_sb = weights.tile([128, G, OUT], bf16, tag="wvo")
            for g in range(G):
                nc.sync.dma_start(wo_sb[:, g, :], wo_d[g])
            ctxT = [
                ctxp.tile([128, N], bf16, tag=f"ctxT_{g}", name=f"ctxT_{g}")
                for g in range(G)
            ]
            for g in range(G):
                if g > 0:
                    kT[g] = emit_kT(g)
                qT = qkv.tile([128, N], bf16, tag="qT", bufs=2, name=f"qT_{g}")
                for qn2 in range(0, QC, 2):
                    pss = [ps_proj.tile([128, 512], f32, tag="pp", name=f"pq{g}{qn2}{j}")
                           for j in range(2)]
                    for kc in range(KC):
                        for j in range(2):
                            nc.tensor.matmul(
                                pss[j][:],
                                wq_sb[:, kc, g * 128:(g + 1) * 128],
                                xtq[kc][:, (qn2 + j) * 512:(qn2 + j + 1) * 512],
                                start=(kc == 0), stop=(kc == KC - 1),
                            )
                    for j in range(2):
                        nc.scalar.add(
                            qT[:, (qn2 + j) * 512:(qn2 + j + 1) * 512],
                            pss[j][:], bq_sb[:, g, :]
                        )

                lo = slice(2 * g * DH, (2 * g + 1) * DH)
                hi = slice((2 * g + 1) * DH, (2 * g + 2) * DH)
                for qc in range(QC):
                    av = ps_av.tile([128, 512], f32, tag="av")
                    eTs = [None] * RC
                    s1 = [None] * KCC2
                    s2 = [None] * 4
                    s3 = [None] * 2
                    for kcc in range(RC):
                        pp = ps_qk.tile([128, 1024], f32, tag="qk")
                        nc.tensor.matmul(
                            pp[:, 0:512],
                            kT[g][0:64, kcc * 128:(kcc + 1) * 128],
                            qT[0:64, qc * 512:(qc + 1) * 512],
                            start=True, stop=True,
                        )
                        nc.tensor.matmul(
                            pp[:, 512:1024],
                            kT[g][64:128, kcc * 128:(kcc + 1) * 128],
                            qT[64:128, qc * 512:(qc + 1) * 512],
                            start=True, stop=True,
                        )
                        eT = expp.tile([128, 1024], bf16, tag="exp")
                        eTs[kcc] = eT
                        nc.scalar.activation(eT[:], pp[:], Exp)
                        nc.tensor.matmul(
                            av[0:64, :],
                            v1[kcc][:, lo],
                            eT[:, 0:512],
                            start=(kcc == 0), stop=(kcc == RC - 1),
                            tile_position=(0, 0),
                        )
                        nc.tensor.matmul(
                            av[64:128, :],
                            v1[kcc][:, hi],
                            eT[:, 512:1024],
                            start=(kcc == 0), stop=(kcc == RC - 1),
                            tile_position=(0, 64),
                        )
                        # denominator tree (bf16 adds, DVE 2x mode)
                        if kcc % 2 == 1:
                            k2 = kcc // 2
                            s1[k2] = treep.tile([128, 1024], bf16, tag="s1",
                                                name=f"s1_{g}{qc}{k2}")
                            nc.vector.tensor_tensor(
                                s1[k2][:], eTs[kcc - 1][:], eT[:], ADD)
                            if k2 % 2 == 1:
                                i = k2 // 2
                                s2[i] = treep.tile([128, 1024], bf16, tag="s2",
                                                   name=f"s2_{g}{qc}{i}")
                                nc.vector.tensor_tensor(
                                    s2[i][:], s1[k2 - 1][:], s1[k2][:], ADD)
                                if i % 2 == 1:
                                    s3[i // 2] = treep.tile(
                                        [128, 1024], bf16, tag="s3",
                                        name=f"s3_{g}{qc}{i // 2}")
                                    nc.vector.tensor_tensor(
                                        s3[i // 2][:], s2[i - 1][:], s2[i][:],
                                        ADD)
                    root = treep.tile([128, 1024], bf16, tag="root",
                                      name=f"root_{g}{qc}")
                    nc.vector.tensor_tensor(root[:], s3[0][:], s3[1][:], ADD)

                    # denominators: col-tiled ones-matmuls both REDUCE over
                    # partitions and REPLICATE the result across partitions:
                    # dd[0:64] = denom_lo (x64 rows), dd[64:128] = denom_hi.
                    dd = ps_av.tile([128, 512], f32, tag="dd")
                    nc.tensor.matmul(dd[0:64, :], ones_sb[:], root[:, 0:512],
                                     start=True, stop=True,
                                     tile_position=(0, 0))
                    nc.tensor.matmul(dd[64:128, :], ones_sb[:],
                                     root[:, 512:1024],
                                     start=True, stop=True,
                                     tile_position=(0, 64))
                    # raw ctx out of PSUM (ScalarE; frees the AV bank)
                    raw = small.tile([128, 512], bf16, tag="raw", bufs=2)
                    nc.scalar.copy(raw[:], av[:])
                    rb = small.tile([128, 512], bf16, tag="rb", bufs=2)
                    with nc.allow_low_precision(
                            reason="softmax denom: positive sums, bf16 ok"):
                        nc.vector.reciprocal(rb[:], dd[:])
                    nc.vector.tensor_tensor(
                        ctxT[g][0:64, qc * 512:(qc + 1) * 512],
                        raw[0:64, :], rb[0:64, :], MULT,
                    )
                    nc.vector.tensor_tensor(
                        ctxT[g][64:128, qc * 512:(qc + 1) * 512],
                        raw[64:128, :], rb[64:128, :], MULT,
                    )

            # ---- output projection: out = ctx @ Wo_slice (biases on host)
            for rc in range(RC):
                pss = [ps_proj.tile([128, 512], f32, tag="pp", name=f"po{rc}{n}")
                       for n in range(2)]
                for g in range(G):
                    for ncol in range(2):
                        nc.tensor.matmul(
                            pss[ncol][:],
                            ctxT[g][:, rc * 128:(rc + 1) * 128],
                            wo_sb[:, g, ncol * 512:(ncol + 1) * 512],
                            start=(g == 0), stop=(g == G - 1),
                        )
                for ncol in range(2):
                    ob = outp.tile([128, 512], bf16, tag="ob")
                    nc.scalar.copy(ob[:], pss[ncol][:])
                    nc.sync.dma_start(
                        out_d[rc * 128:(rc + 1) * 128,
                              ncol * 512:(ncol + 1) * 512], ob[:]
                    )

    nc.compile()
    return nc


def _get_module(reps=1):
    key = ("nc", reps)
    if key not in _cache:
        _cache[key] = _build_module(reps)
    return _cache[key]


def _get_runner(reps=1, donate=True):
    """Build the PJRT executable once (mirrors bass2jax.run_bass_via_pjrt) and
    return a callable in_maps -> list of per-core output dicts."""
    rkey = ("runner", reps, donate)
    if rkey in _cache:
        return _cache[rkey]

    import jax
    import numpy as np
    import concourse.mybir as mybir
    from concourse import bass2jax
    from jax.sharding import Mesh, PartitionSpec
    from jax.experimental.shard_map import shard_map

    nc = _get_module(reps)
    bass2jax.install_neuronx_cc_hook()

    partition_name = nc.partition_id_tensor.name if nc.partition_id_tensor else None
    in_names, out_names, out_avals, zero_outs = [], [], [], []
    for alloc in nc.m.functions[0].allocations:
        if not isinstance(alloc, mybir.MemoryLocationSet):
            continue
        name = alloc.memorylocations[0].name
        if alloc.kind == "ExternalInput":
            if name != partition_name:
                in_names.append(name)
        elif alloc.kind == "ExternalOutput":
            shape = tuple(alloc.tensor_shape)
            dtype = mybir.dt.np(alloc.dtype)
            out_names.append(name)
            out_avals.append(jax.core.ShapedArray(shape, dtype))
            zero_outs.append(np.zeros(shape, dtype))
    n_params = len(in_names)
    n_outs = len(out_avals)
    all_in_names = list(in_names) + list(out_names)
    if partition_name is not None:
        all_in_names.append(partition_name)
    donate_idx = tuple(range(n_params, n_params + n_outs))

    def _body(*args):
        operands = list(args)
        if partition_name is not None:
            operands.append(bass2jax.partition_id_tensor())
        outs = bass2jax._bass_exec_p.bind(
            *operands,
            out_avals=tuple(out_avals),
            in_names=tuple(all_in_names),
            out_names=tuple(out_names),
            lowering_input_output_aliases=(),
            sim_require_finite=True,
            sim_require_nnan=True,
            nc=nc,
        )
        return tuple(outs)

    devices = jax.devices()[:NCORES]
    mesh = Mesh(np.asarray(devices), ("core",))
    in_specs = (PartitionSpec("core"),) * (n_params + n_outs)
    out_specs = (PartitionSpec("core"),) * n_outs
    sharded = jax.jit(
        shard_map(_body, mesh=mesh, in_specs=in_specs, out_specs=out_specs,
                  check_rep=False),
        donate_argnums=(donate_idx if donate else ()), keep_unused=True,
    )

    def run(in_maps):
        concat_in = [
            np.concatenate([np.asarray(in_maps[c][name]) for c in range(NCORES)], axis=0)
            for name in in_names
        ]
        concat_zeros = [
            np.zeros((NCORES * z.shape[0], *z.shape[1:]), z.dtype) for z in zero_outs
        ]
        out_arrs = sharded(*concat_in, *concat_zeros)
        return [
            {
                name: np.asarray(out_arrs[i]).reshape(NCORES, *out_avals[i].shape)[c]
                for i, name in enumerate(out_names)
            }
            for c in range(NCORES)
        ]

    run.in_names = in_names
    run.out_names = out_names
    run.out_avals = out_avals
    run.zero_outs = zero_outs
    run.sharded = sharded
    _cache[rkey] = run
    return run


def _shard_inputs(key, value, query, Wk, bk, Wv, bv, Wq, bq, Wo, bo):
    bf = ml_dtypes.bfloat16
    f32 = np.float32
    scale = 1.0 / np.sqrt(np.float32(DH))

    xt = {}  # per batch transposed inputs
    for b in range(B):
        xt[b] = {
            "q": np.ascontiguousarray(query[b].T).reshape(KC, 128, N).astype(bf),
            "k": np.ascontiguousarray(key[b].T).reshape(KC, 128, N).astype(bf),
            "v": np.ascontiguousarray(value[b].T).reshape(KC, 128, N).astype(bf),
        }

    in_maps = []
    for c in range(NCORES):
        b, half = divmod(c, 2)
        cols = slice(half * 512, (half + 1) * 512)
        in_maps.append({
            "xt_q": xt[b]["q"],
            "xt_k": xt[b]["k"],
            "xt_v": xt[b]["v"],
            "wq": np.ascontiguousarray(Wq[:, cols] * scale).reshape(KC, 128, 512).astype(bf),
            "wk": np.ascontiguousarray(Wk[:, cols]).reshape(KC, 128, 512).astype(bf),
            "wv": np.ascontiguousarray(Wv[:, cols]).reshape(KC, 128, 512).astype(bf),
            "wo": np.ascontiguousarray(Wo[cols, :]).reshape(G, 128, OUT).astype(bf),
            "bq2": (bq[cols] * scale).reshape(G, 128, 1).astype(f32),
            "bk2": bk[cols].reshape(G, 128, 1).astype(f32),
        })
    return in_maps


def kernel(key, value, query, Wk, bk, Wv, bv, Wq, bq, Wo, bo):
    key, value, query = np.asarray(key), np.asarray(value), np.asarray(query)
    Wk, bk, Wv, bv = np.asarray(Wk), np.asarray(bk), np.asarray(Wv), np.asarray(bv)
    Wq, bq, Wo, bo = np.asarray(Wq), np.asarray(bq), np.asarray(Wo), np.asarray(bo)

    run = _get_runner()
    in_maps = _shard_inputs(key, value, query, Wk, bk, Wv, bv, Wq, bq, Wo, bo)
    results = run(in_maps)
    # bv commutes through softmax (attention rows sum to 1): fold into bias.
    bo_total = (bo.astype(np.float64) + bv.astype(np.float64) @ Wo.astype(np.float64)
                ).astype(np.float32)
    out = np.empty((B, N, OUT), np.float32)
    for b in range(B):
        out[b] = (results[2 * b]["out"].astype(np.float32)
                  + results[2 * b + 1]["out"].astype(np.float32)
                  + bo_total)
    return out
